# revision 1
# baseline (speedup 1.0000x reference)
"""ChebGCN (K=2, 2-layer) on 8 Trainium2 NeuronCores.

Full inputs in, full output out. Internally:
  - nodes partitioned by id across 8 cores (graph-parallel, per sharding hint)
  - per-core dest nodes bin-packed into 49 blocks x 128 slots (balanced)
  - messages reduced to post-weight space first: tx1@W1 == segsum(norm * (x@W1)[col])
  - gather tables in HBM, rows = 256B; dma_gather (int16 idx => lo/hi table halves)
  - scatter-add via one-hot matmuls accumulating in PSUM per dest block
  - layer-2 source features exchanged with an AllGather collective
Host does sharding prep (sort/pad/index building) and output reassembly only.
"""
import sys

for _p in ("/opt/trn_rl_repo",):
    if _p not in sys.path:
        sys.path.insert(0, _p)

import numpy as np
import concourse.bass as bass
import concourse.bacc as bacc
import concourse.mybir as mybir
import concourse.tile as tile
from concourse.bass_utils import run_bass_kernel_spmd

N = 50000
E = 800000
NCORE = 8
SH = 6250           # nodes per core
NB = 49             # dest blocks per core
P = 128
TPC = NB * P        # 6272 table rows per core
TR = NCORE * TPC    # 50176 table rows
HALF = 32768
F_IN, F_HID, F_OUT = 96, 64, 40
FP = 64             # padded feature dim (256B rows)
G = 16              # chunks per dma_gather group (overridden per attempt)

dt = mybir.dt


# ----------------------------------------------------------------- host prep
def _bin_pack_blocks(deg_local):
    order = np.argsort(-deg_local, kind="stable")
    loads = np.zeros(NB, np.int64)
    counts = np.zeros(NB, np.int32)
    slot = np.full(SH, -1, np.int64)
    big = np.iinfo(np.int64).max
    for l in order:
        b = int(np.argmin(np.where(counts < P, loads, big)))
        slot[l] = b * P + counts[b]
        counts[b] += 1
        loads[b] += deg_local[l]
    return slot


def _build_plan(edge_index):
    row = np.asarray(edge_index[0], np.int64)
    col = np.asarray(edge_index[1], np.int64)
    deg = np.bincount(row, minlength=N).astype(np.float32)
    dis = np.where(deg > 0, 1.0 / np.sqrt(np.maximum(deg, 1e-12)), 0.0).astype(np.float32)
    norm = (-dis[row] * dis[col]).astype(np.float32)

    slot_of_node = np.zeros(N, np.int64)
    pi_inv = np.full((NCORE, TPC), -1, np.int64)
    for c in range(NCORE):
        deg_local = deg[c * SH:(c + 1) * SH].astype(np.int64)
        slot = _bin_pack_blocks(deg_local)
        slot_of_node[c * SH:(c + 1) * SH] = slot
        pi_inv[c, slot] = np.arange(c * SH, (c + 1) * SH)

    own = np.arange(N) // SH
    s = slot_of_node
    table_row = own * TPC + (s % P) * NB + (s // P)

    cd = row // SH
    src_row = table_row[col]
    half = (src_row >= HALF).astype(np.int64)
    dst_slot = slot_of_node[row]

    cores = []
    maxcl = maxch = 0
    for c in range(NCORE):
        m = cd == c
        er = np.stack(
            [dst_slot[m], half[m], src_row[m],
             norm[m].view(np.int32).astype(np.int64)], axis=1)
        db = er[:, 0] // P
        er = er[np.lexsort((er[:, 2], er[:, 1], db))]
        db = er[:, 0] // P
        cores.append(er)
        for b in range(NB):
            mb = db == b
            nlo = int((er[mb, 1] == 0).sum())
            nhi = int((er[mb, 1] == 1).sum())
            maxcl = max(maxcl, -(-nlo // P))
            maxch = max(maxch, -(-nhi // P))
    CL, CH = max(maxcl, 1), max(maxch, 1)
    NLO, NHI = NB * CL, NB * CH

    def wrap_idx(v):
        n = len(v)
        a = np.zeros((16, n // 16), np.int16)
        a[np.arange(n) % 16, np.arange(n) // 16] = v
        return np.tile(a, (8, 1))

    plans = []
    for c in range(NCORE):
        er = cores[c]
        db = er[:, 0] // P
        arrs = {}
        for h, C in ((0, CL), (1, CH)):
            nn = NB * C * P
            idx = np.zeros(nn, np.int64)
            nrm = np.zeros(nn, np.float32)
            dp = np.zeros(nn, np.int64)
            for b in range(NB):
                mb = (db == b) & (er[:, 1] == h)
                sub = er[mb]
                n = len(sub)
                o = b * C * P
                idx[o:o + n] = sub[:, 2] - h * HALF
                nrm[o:o + n] = sub[:, 3].astype(np.int32).view(np.float32)
                dp[o:o + n] = sub[:, 0] % P
            key = "lo" if h == 0 else "hi"
            arrs["idx_" + key] = wrap_idx(idx.astype(np.int16))
            arrs["nrm_" + key] = np.ascontiguousarray(nrm.reshape(-1, P).T)
            arrs["dp_" + key] = np.ascontiguousarray(
                dp.reshape(-1, P).T.astype(np.float32))
        plans.append(arrs)

    return dict(plans=plans, pi_inv=pi_inv, CL=CL, CH=CH, NLO=NLO, NHI=NHI)


def _build_xt(x, pi_inv):
    xp = np.zeros((TR, F_IN), np.float32)
    for c in range(NCORE):
        valid = pi_inv[c] >= 0
        xp[c * TPC:(c + 1) * TPC][valid] = x[pi_inv[c][valid]]
    return np.ascontiguousarray(xp.T)  # [96, TR] slot-major (cn, b, p)


# ------------------------------------------------------------------ device
def _build_graph(CL, CH):
    NLO, NHI = NB * CL, NB * CH
    # two SWDGE queues: alternate dma_gather descriptor generation across
    # both Q7 core pairs; single_packet=False keeps ring packets <=64 descs.
    nc = bacc.Bacc("TRN2", target_bir_lowering=False, num_devices=NCORE,
                   num_swdge_queues=2 if G > 4 else 1)

    f32, i16 = dt.float32, dt.int16
    xt_all = nc.dram_tensor("xt_all", [F_IN, TR], f32, kind="ExternalInput")
    xt_own = nc.dram_tensor("xt_own", [F_IN, TPC], f32, kind="ExternalInput")
    w10 = nc.dram_tensor("w10", [F_IN, F_HID], f32, kind="ExternalInput")
    w11 = nc.dram_tensor("w11", [F_IN, F_HID], f32, kind="ExternalInput")
    w20p = nc.dram_tensor("w20p", [F_HID, FP], f32, kind="ExternalInput")
    w21p = nc.dram_tensor("w21p", [F_HID, FP], f32, kind="ExternalInput")
    b1r = nc.dram_tensor("b1r", [1, F_HID], f32, kind="ExternalInput")
    b2r = nc.dram_tensor("b2r", [1, FP], f32, kind="ExternalInput")
    onesr = nc.dram_tensor("onesr", [1, P], f32, kind="ExternalInput")
    ident = nc.dram_tensor("ident", [P, P], f32, kind="ExternalInput")
    iota = nc.dram_tensor("iota", [P, P], f32, kind="ExternalInput")
    idx_lo = nc.dram_tensor("idx_lo", [P, NLO * 8], i16, kind="ExternalInput")
    idx_hi = nc.dram_tensor("idx_hi", [P, NHI * 8], i16, kind="ExternalInput")
    nrm_lo = nc.dram_tensor("nrm_lo", [P, NLO], f32, kind="ExternalInput")
    nrm_hi = nc.dram_tensor("nrm_hi", [P, NHI], f32, kind="ExternalInput")
    dp_lo = nc.dram_tensor("dp_lo", [P, NLO], f32, kind="ExternalInput")
    dp_hi = nc.dram_tensor("dp_hi", [P, NHI], f32, kind="ExternalInput")
    out = nc.dram_tensor("out", [P, NB, F_OUT], f32, kind="ExternalOutput")

    y1_tab = nc.dram_tensor("y1_tab", [TR, FP], f32, kind="Internal")
    z_bounce = nc.dram_tensor("z_bounce", [TPC, FP], f32, kind="Internal")
    z_full = nc.dram_tensor("z_full", [TR, FP], f32, kind="Internal")

    with tile.TileContext(nc) as tc:
        with (
            tc.tile_pool(name="const", bufs=1) as cpool,
            tc.tile_pool(name="persist", bufs=1) as ppool,
            tc.tile_pool(name="hsp", bufs=2) as hsp,
            tc.tile_pool(name="psT", bufs=2, space="PSUM") as psT,
            tc.tile_pool(name="psZ", bufs=2, space="PSUM") as psZ,
        ):
            # ---- constants / persistent loads
            def load(pool, src, shape, dtype=f32, tag=None):
                t = pool.tile(shape, dtype, tag=tag)
                nc.sync.dma_start(t[:], src[:])
                return t

            w10_t = load(cpool, w10, [F_IN, F_HID], tag="w10")
            w11_t = load(cpool, w11, [F_IN, F_HID], tag="w11")
            w20_t = load(cpool, w20p, [F_HID, FP], tag="w20")
            w21_t = load(cpool, w21p, [F_HID, FP], tag="w21")
            b1_t = load(cpool, b1r, [1, F_HID], tag="b1")
            b2_t = load(cpool, b2r, [1, FP], tag="b2")
            ones_t = load(cpool, onesr, [1, P], tag="ones")
            id_t = load(cpool, ident, [P, P], tag="ident")
            io_t = load(cpool, iota, [P, P], tag="iota")
            ixlo_t = load(cpool, idx_lo, [P, NLO * 8], i16, tag="ixlo")
            ixhi_t = load(cpool, idx_hi, [P, NHI * 8], i16, tag="ixhi")
            nlo_t = load(cpool, nrm_lo, [P, NLO], tag="nlo")
            nhi_t = load(cpool, nrm_hi, [P, NHI], tag="nhi")
            dlo_t = load(cpool, dp_lo, [P, NLO], tag="dlo")
            dhi_t = load(cpool, dp_hi, [P, NHI], tag="dhi")
            xo_t = load(ppool, xt_own, [F_IN, TPC], tag="xown")

            hT = ppool.tile([F_HID, TPC], f32, tag="hT")
            z_stage = ppool.tile([P, NB, FP], f32, tag="zst")
            out_stage = ppool.tile([P, NB, F_OUT], f32, tag="ost")

            # ---- phase A: y1 = x @ W1_1 for all nodes -> y1_tab (p-major)
            with (
                tc.tile_pool(name="xa2", bufs=2) as xa,
                tc.tile_pool(name="ya2", bufs=2) as ya,
                tc.tile_pool(name="psA", bufs=4, space="PSUM") as psA,
            ):
                BPH = min(25, NB)  # blocks per xt slice (25+24)
                for cn in range(NCORE):
                    yst = ya.tile([P, NB, FP], f32, tag="yst")
                    b0 = 0
                    for hf, nblk in ((0, BPH), (1, NB - BPH)):
                        if nblk == 0:
                            continue
                        cols = nblk * P
                        xs = xa.tile([F_IN, BPH * P], f32, tag="xs")
                        nc.sync.dma_start(
                            xs[:, :cols],
                            xt_all[:, cn * TPC + b0 * P: cn * TPC + (b0 + nblk) * P])
                        for bb in range(nblk):
                            b = b0 + bb
                            ps = psA.tile([P, F_HID], f32, tag="psy")
                            nc.tensor.matmul(
                                out=ps[:], lhsT=xs[:, bb * P:(bb + 1) * P],
                                rhs=w11_t[:], start=True, stop=True)
                            if b % 2 == 0:
                                nc.vector.tensor_copy(yst[:, b, :], ps[:])
                            else:
                                nc.scalar.copy(yst[:, b, :], ps[:])
                        b0 += nblk
                    nc.sync.dma_start(
                        y1_tab[cn * TPC:(cn + 1) * TPC, :].rearrange(
                            "(p k) f -> p k f", p=P),
                        yst[:])

            # ---- spmm pass (shared for both layers)
            def spmm_pass(tab, evict, sfx):
                NGLO = -(-NLO // G)
                NGHI = -(-NHI // G)
                with (
                    tc.tile_pool(name="mlo" + sfx, bufs=3) as mlo,
                    tc.tile_pool(name="mhi" + sfx, bufs=3) as mhi,
                    tc.tile_pool(name="ohp" + sfx, bufs=3) as ohp,
                    tc.tile_pool(name="psX" + sfx, bufs=4, space="PSUM") as psX,
                ):
                    glo_tiles = [None] * NGLO
                    ghi_tiles = [None] * NGHI

                    def get_group(is_lo, g):
                        tiles = glo_tiles if is_lo else ghi_tiles
                        if tiles[g] is not None:
                            return tiles[g]
                        NT, ixt, nt, pool, tag = (
                            (NLO, ixlo_t, nlo_t, mlo, "mlo") if is_lo else
                            (NHI, ixhi_t, nhi_t, mhi, "mhi"))
                        base = tab[0:HALF, :] if is_lo else tab[HALF:TR, :]
                        ncg = min(G, NT - g * G)
                        ni = ncg * P
                        m = pool.tile([P, G, FP], f32, tag=tag)
                        if G > 4:
                            nc.gpsimd.dma_gather(
                                m[:, :ncg, :], base,
                                ixt[:, g * G * 8:(g * G + ncg) * 8],
                                ni, ni, FP, single_packet=False,
                                queue_num=(g + (0 if is_lo else 1)) % 2)
                        else:
                            nc.gpsimd.dma_gather(
                                m[:, :ncg, :], base,
                                ixt[:, g * G * 8:(g * G + ncg) * 8],
                                ni, ni, FP)
                        nc.vector.tensor_tensor(
                            out=m[:, :ncg, :],
                            in0=m[:, :ncg, :],
                            in1=nt[:, g * G:g * G + ncg].to_broadcast([P, ncg, FP]),
                            op=mybir.AluOpType.mult)
                        tiles[g] = m
                        return m

                    for b in range(NB):
                        ps = psX.tile([P, FP], f32, tag="acc")
                        # one-hot tiles for this block
                        oh_lo = ohp.tile([P, CL * P], f32, tag="ohlo")
                        nc.vector.tensor_tensor(
                            out=oh_lo[:].rearrange("p (c j) -> p c j", c=CL),
                            in0=dlo_t[:, b * CL:(b + 1) * CL].to_broadcast([P, CL, P]),
                            in1=bass.AP(io_t[:].tensor, io_t[:].offset,
                                        [io_t[:].ap[0], [0, CL], [1, P]]),
                            op=mybir.AluOpType.is_equal)
                        oh_hi = ohp.tile([P, CH * P], f32, tag="ohhi")
                        nc.vector.tensor_tensor(
                            out=oh_hi[:].rearrange("p (c j) -> p c j", c=CH),
                            in0=dhi_t[:, b * CH:(b + 1) * CH].to_broadcast([P, CH, P]),
                            in1=bass.AP(io_t[:].tensor, io_t[:].offset,
                                        [io_t[:].ap[0], [0, CH], [1, P]]),
                            op=mybir.AluOpType.is_equal)
                        for j in range(CL):
                            q = b * CL + j
                            m = get_group(True, q // G)
                            nc.tensor.matmul(
                                out=ps[:], lhsT=oh_lo[:, j * P:(j + 1) * P],
                                rhs=m[:, q % G, :], start=(j == 0), stop=False)
                        for j in range(CH):
                            q = b * CH + j
                            m = get_group(False, q // G)
                            nc.tensor.matmul(
                                out=ps[:], lhsT=oh_hi[:, j * P:(j + 1) * P],
                                rhs=m[:, q % G, :], start=False, stop=False)
                        evict(b, ps)

            # ---- layer 1 eviction: h block
            def evict_l1(b, ps):
                nc.tensor.matmul(out=ps[:], lhsT=xo_t[:, b * P:(b + 1) * P],
                                 rhs=w10_t[:], start=False, stop=False)
                nc.tensor.matmul(out=ps[:], lhsT=ones_t[:], rhs=b1_t[:],
                                 start=False, stop=True)
                hs = hsp.tile([P, F_HID], f32, tag="hs")
                nc.scalar.activation(hs[:], ps[:], mybir.ActivationFunctionType.Relu)
                pt = psT.tile([F_HID, P], f32, tag="pt")
                nc.tensor.transpose(out=pt[:], in_=hs[:], identity=id_t[:])
                nc.vector.tensor_copy(hT[:, b * P:(b + 1) * P], pt[:])
                pz = psZ.tile([P, FP], f32, tag="pz")
                nc.tensor.matmul(out=pz[:], lhsT=hT[:, b * P:(b + 1) * P],
                                 rhs=w21_t[:], start=True, stop=True)
                nc.scalar.copy(z_stage[:, b, :], pz[:])

            spmm_pass(y1_tab, evict_l1, "a")

            # ---- exchange
            nc.sync.dma_start(
                z_bounce[:].rearrange("(p k) f -> p k f", p=P), z_stage[:])
            nc.gpsimd.collective_compute(
                "AllGather", mybir.AluOpType.bypass,
                replica_groups=[list(range(NCORE))],
                ins=[z_bounce[:].opt()],
                outs=[z_full[:].opt()],
            )

            # ---- layer 2 eviction: out block
            def evict_l2(b, ps):
                nc.tensor.matmul(out=ps[:], lhsT=hT[:, b * P:(b + 1) * P],
                                 rhs=w20_t[:], start=False, stop=False)
                nc.tensor.matmul(out=ps[:], lhsT=ones_t[:], rhs=b2_t[:],
                                 start=False, stop=True)
                if b % 2 == 0:
                    nc.scalar.copy(out_stage[:, b, :], ps[:, :F_OUT])
                else:
                    nc.vector.tensor_copy(out_stage[:, b, :], ps[:, :F_OUT])

            spmm_pass(z_full, evict_l2, "b")

            nc.sync.dma_start(out[:], out_stage[:])

    nc.compile()
    return nc


_GRAPH_CACHE = {}


def kernel(x, edge_index, W1_0, W1_1, b1, W2_0, W2_1, b2):
    x = np.asarray(x, np.float32)
    W1_0 = np.asarray(W1_0, np.float32)
    W1_1 = np.asarray(W1_1, np.float32)
    b1 = np.asarray(b1, np.float32)
    W2_0 = np.asarray(W2_0, np.float32)
    W2_1 = np.asarray(W2_1, np.float32)
    b2 = np.asarray(b2, np.float32)

    plan = _build_plan(edge_index)
    CL, CH = plan["CL"], plan["CH"]

    xt = _build_xt(x, plan["pi_inv"])
    w20p = np.zeros((F_HID, FP), np.float32); w20p[:, :F_OUT] = W2_0
    w21p = np.zeros((F_HID, FP), np.float32); w21p[:, :F_OUT] = W2_1
    b2p = np.zeros((1, FP), np.float32); b2p[0, :F_OUT] = b2
    ident = np.eye(P, dtype=np.float32)
    iota = np.tile(np.arange(P, dtype=np.float32), (P, 1))
    ones = np.ones((1, P), np.float32)

    common = dict(
        xt_all=xt, w10=W1_0, w11=W1_1,
        w20p=w20p, w21p=w21p,
        b1r=b1.reshape(1, F_HID), b2r=b2p,
        onesr=ones, ident=ident, iota=iota,
    )
    in_maps = []
    for c in range(NCORE):
        pl = plan["plans"][c]
        m = dict(common)
        m["xt_own"] = np.ascontiguousarray(xt[:, c * TPC:(c + 1) * TPC])
        m["idx_lo"] = pl["idx_lo"]; m["idx_hi"] = pl["idx_hi"]
        m["nrm_lo"] = pl["nrm_lo"]; m["nrm_hi"] = pl["nrm_hi"]
        m["dp_lo"] = pl["dp_lo"]; m["dp_hi"] = pl["dp_hi"]
        in_maps.append(m)

    global G
    res = None
    last_exc = None
    for g_try in (16, 4, 2):
        G = g_try
        key = (CL, CH, g_try)
        try:
            if key not in _GRAPH_CACHE:
                _GRAPH_CACHE[key] = _build_graph(CL, CH)
            res = run_bass_kernel_spmd(
                _GRAPH_CACHE[key], in_maps, core_ids=list(range(NCORE)))
            break
        except Exception as e:  # noqa: BLE001 - retry with safer gather size
            last_exc = e
            import time as _t
            _t.sleep(10)
    if res is None:
        raise last_exc
    kernel.last_result = res

    out_full = np.zeros((N, F_OUT), np.float32)
    pi_inv = plan["pi_inv"]
    for c in range(NCORE):
        o = res.results[c]["out"].transpose(1, 0, 2).reshape(TPC, F_OUT)
        valid = pi_inv[c] >= 0
        out_full[pi_inv[c][valid]] = o[valid]
    return out_full

